# revision 19
# baseline (speedup 1.0000x reference)
"""GVSL loss (NCC + MSE + smoothness) as a distributed Bass kernel on 8 TRN2 cores.

Sharding: batch(2) x depth-quarters(4) = 8 shards; each core owns a 32-deep
output slab (+4-voxel halo for the 9^3 box filter).

NCC box filter strategy (per var in {I, J, I^2, J^2, IJ}):
  pass1 (PE):  per d-row matmul(lhsT=V_d[h,w], rhs=BandH[h,h']) -> PSUM [w, h]
               = H-box + transpose in one shot (fp16, FD=128, LDW pipelined)
  evac1:       PSUM -> SBUF fp16 YT [w, (d, h)]
  pass2 (PE):  stationary BandW (scaled); 3 d-shifted FD=512 matmuls accumulate
               -> t3[r] = Z[r]+Z[r+1]+Z[r+2] (W-box + D-triple), PSUM
  evac2:       PSUM -> SBUF fp16 T3 [w', (r, h)]
  D-final(DVE): S = t3[d] + t3[d+3] + t3[d+6]  (9-window box done)

Scaling: quadratic vars (II/JJ/IJ) get BandW*s (s=fp16(1/27)); linear vars
(I/J) get BandW*t with t^2 = s/729, so cross = B'IJ - B'I*B'J and
var = B'II - B'I^2 need no scalar coefficient (all plain tensor ops), and
everything stays in fp16 range. eps scales as 1e-5*s^2.

The depth range is processed in two phases (A: do 0..15, B: do 16..31) so the
cc math for A overlaps phase B's box-filter work.
"""

import os
import sys

for _p in ("/opt/trn_rl_repo",):
    if _p not in sys.path:
        sys.path.insert(0, _p)

import numpy as np

import concourse.bass as bass
import concourse.tile as tile
from concourse import bacc, mybir
from concourse.bass_utils import run_bass_kernel_spmd

F32 = mybir.dt.float32
F16 = mybir.dt.float16
AF = mybir.ActivationFunctionType
ALU = mybir.AluOpType

HP = 128
W = 128
D_FULL = 128
DQ = 32
D_IN = DQ + 8     # 40 slab rows incl halo
YT_R = 42         # yt rows incl zero tail (pass2 k=9 s=2 reads rows 38..41)
T3_R = 40
FLOW_D = DQ + 1   # 33

N_IN = D_IN * W           # 5120
N_YT = YT_R * HP          # 5376
N_T3 = T3_R * HP          # 5120
N_BOX = DQ * HP           # 4096
N_RECON = DQ * W          # 4096
N_FLOW_C = FLOW_D * W     # 4224

S16 = float(np.float16(1.0 / 27.0))          # quadratic-var scale
TLIN = float(np.float16(np.sqrt(S16 / 729.0)))  # linear-var scale
EPS_S = 1e-5 * S16 * S16

COL_CC = 0     # +4
COL_MSE = 4
COL_DX = 5     # +3
COL_DZ = 8     # +3
COL_DY = 11    # +12
ACC_W = 24

VARS = ("J", "I", "II", "JJ", "IJ")

_CACHE = {}


def _patch_act_tables():
    """Reorder activation-table sets so the one containing ln+exp+square+copy
    is preferred, avoiding table reloads between Ln and Exp/Square."""
    from concourse import hw_specs

    if getattr(hw_specs, "_gvsl_patched", False):
        return
    orig = hw_specs.get_activation_tables

    def patched(arch):
        t = dict(orig(arch))
        key = "natural_log_exp_and_others"
        if key in t:
            t = {key: t[key], **{k: v for k, v in t.items() if k != key}}
        return t

    hw_specs.get_activation_tables = patched
    bacc.get_activation_tables = patched
    hw_specs._gvsl_patched = True


def _build_program():
    nc = bacc.Bacc("TRN2", target_bir_lowering=False, debug=False, num_devices=8)

    d_inI = nc.dram_tensor("inI", [HP, N_IN], F16, kind="ExternalInput").ap()
    d_inJ = nc.dram_tensor("inJ", [HP, N_IN], F16, kind="ExternalInput").ap()
    d_recon = nc.dram_tensor("recon", [HP, N_RECON], F16, kind="ExternalInput").ap()
    d_flow = nc.dram_tensor("flow", [HP, 3 * N_FLOW_C], F16, kind="ExternalInput").ap()
    d_bandh = nc.dram_tensor("bandh", [HP, HP], F16, kind="ExternalInput").ap()
    d_bandq = nc.dram_tensor("bandq", [HP, HP], F16, kind="ExternalInput").ap()
    d_bandl = nc.dram_tensor("bandl", [HP, HP], F16, kind="ExternalInput").ap()
    d_bidiag = nc.dram_tensor("bidiag", [HP, HP - 1], F16, kind="ExternalInput").ap()
    d_out = nc.dram_tensor("out", [HP, ACC_W], F32, kind="ExternalOutput").ap()

    from contextlib import ExitStack

    with tile.TileContext(nc) as tc, ExitStack() as es:
        pp = es.enter_context(tc.tile_pool(name="persist", bufs=1))
        prp = es.enter_context(tc.tile_pool(name="prodp", bufs=3))
        ytp = es.enter_context(tc.tile_pool(name="ytp", bufs=2))
        t3p = es.enter_context(tc.tile_pool(name="t3p", bufs=2))
        bxp = es.enter_context(tc.tile_pool(name="boxp", bufs=1))
        scp = es.enter_context(tc.tile_pool(name="scrp", bufs=2))
        flp = es.enter_context(tc.tile_pool(name="flowscr", bufs=2))
        ps1 = es.enter_context(tc.tile_pool(name="psum1", bufs=2, space="PSUM"))
        ps2 = es.enter_context(tc.tile_pool(name="psum2", bufs=2, space="PSUM"))

        acc = pp.tile([HP, ACC_W], F32, tag="acc", name="acc")[:]
        eps_ap = pp.tile([HP, 1], F32, tag="epsc", name="epsc")[:]
        nc.gpsimd.memset(eps_ap, EPS_S)

        bandh = pp.tile([HP, HP], F16, tag="bandh", name="bandh")[:]
        bandq = pp.tile([HP, HP], F16, tag="bandq", name="bandq")[:]
        bandl = pp.tile([HP, HP], F16, tag="bandl", name="bandl")[:]
        bidiag = pp.tile([HP, HP - 1], F16, tag="bidiag", name="bidiag")[:]
        inI = pp.tile([HP, N_IN], F16, tag="inI", name="inI")[:]
        inJ = pp.tile([HP, N_IN], F16, tag="inJ", name="inJ")[:]
        recon = pp.tile([HP, N_RECON], F16, tag="recon", name="recon")[:]
        flow = pp.tile([HP, 3 * N_FLOW_C], F16, tag="flow", name="flow")[:]

        nc.sync.dma_start(out=bandh, in_=d_bandh)
        nc.sync.dma_start(out=bandq, in_=d_bandq)
        nc.sync.dma_start(out=bandl, in_=d_bandl)
        nc.sync.dma_start(out=bidiag, in_=d_bidiag)
        NQ = N_IN // 4
        for c in range(4):
            nc.sync.dma_start(out=inJ[:, NQ * c : NQ * (c + 1)],
                              in_=d_inJ[:, NQ * c : NQ * (c + 1)])
        for c in range(4):
            nc.sync.dma_start(out=inI[:, NQ * c : NQ * (c + 1)],
                              in_=d_inI[:, NQ * c : NQ * (c + 1)])
        nc.sync.dma_start(out=recon, in_=d_recon)
        nc.sync.dma_start(out=flow, in_=d_flow)

        inI_r = inI.rearrange("p (d w) -> p d w", w=W)
        inJ_r = inJ.rearrange("p (d w) -> p d w", w=W)

        evac_ct = [0]

        def evac(dst, src):
            # rotate PSUM->SBUF copies across DVE / ACT / GPSIMD
            if evac_ct[0] % 2 == 0:
                nc.vector.tensor_copy(dst, src)
            else:
                nc.scalar.copy(dst, src)
            evac_ct[0] += 1

        def product(v):
            prod = prp.tile([HP, N_IN], F16, tag="prod", name=f"prod{v}")[:]
            if v == "II":
                nc.gpsimd.tensor_mul(prod, inI, inI)
            elif v == "JJ":
                nc.vector.tensor_mul(prod, inJ, inJ)
            else:
                nc.vector.tensor_mul(prod, inI, inJ)
            return prod.rearrange("p (d w) -> p d w", w=W)

        def pass1_chunks(v, src_r, yt_r):
            def mk(g0):
                def emit():
                    pst = ps1.tile([HP, 1024], F32, tag="ps1", name="ps1")[:]
                    for q in range(8):
                        nc.tensor.matmul(
                            pst[:, 128 * q : 128 * (q + 1)],
                            src_r[:, g0 + q, :],
                            bandh,
                            start=True,
                            stop=True,
                        )
                    dst = yt_r[:, g0 : g0 + 8, :].rearrange("p d h -> p (d h)")
                    evac(dst, pst)
                return emit
            return [mk(g0) for g0 in range(0, D_IN, 8)]

        def pass2_chunks(v, yt_r, t3_r):
            bw = bandl if v in ("I", "J") else bandq
            def mk(k0):
                def emit():
                    pst = ps2.tile([HP, 1024], F32, tag="ps2", name="ps2")[:]
                    for ki in range(2):
                        k = k0 + ki
                        for s in range(3):
                            rhs = yt_r[:, 4 * k + s : 4 * k + s + 4, :].rearrange(
                                "p d h -> p (d h)"
                            )
                            nc.tensor.matmul(
                                pst[:, 512 * ki : 512 * (ki + 1)],
                                bw,
                                rhs,
                                start=(s == 0),
                                stop=(s == 2),
                            )
                    dst = t3_r[:, 4 * k0 : 4 * k0 + 8, :].rearrange("p r h -> p (r h)")
                    evac(dst, pst)
                return emit
            return [mk(k0) for k0 in range(0, 10, 2)]

        def d_final(v, t3_r, gps=False):
            eng = nc.gpsimd if gps else nc.vector
            B = bxp.tile([HP, N_BOX], F16, tag=f"box{v}", name=f"box{v}")[:]
            B_r = B.rearrange("p (do h) -> p do h", h=HP)
            eng.tensor_add(B_r, t3_r[:, 0:DQ, :], t3_r[:, 3 : 3 + DQ, :])
            eng.tensor_add(B_r, B_r, t3_r[:, 6 : 6 + DQ, :])
            return B

        flow_r = flow.rearrange("p (c d w) -> p c d w", c=3, w=W)

        def flow_dy(c):
            fc_flat = flow_r[:, c].rearrange("p d w -> p (d w)")
            for half in range(2):
                pst = ps1.tile([HP, 1024], F32, tag="ps1", name="dy")[:]
                for j in range(2):
                    off = 2048 * half + 1024 * j
                    for jj in range(2):
                        nc.tensor.matmul(
                            pst[0 : HP - 1, 512 * jj : 512 * (jj + 1)],
                            bidiag,
                            fc_flat[:, off + 512 * jj : off + 512 * (jj + 1)],
                            start=True,
                            stop=True,
                        )
                    col = COL_DY + 4 * c + 2 * half + j
                    nc.scalar.activation(
                        pst[0 : HP - 1, :],
                        pst[0 : HP - 1, :],
                        AF.Square,
                        accum_out=acc[0 : HP - 1, col : col + 1],
                    )

        def flow_dxz(c):
            fc_flat = flow_r[:, c].rearrange("p d w -> p (d w)")
            db = flp.tile([HP, N_RECON], F16, tag="fscr", name=f"dx{c}")[:]
            nc.vector.tensor_sub(
                db, fc_flat[:, 1 : 1 + N_RECON], fc_flat[:, 0:N_RECON]
            )
            db_r = db.rearrange("p (d w) -> p d w", w=W)
            nc.scalar.activation(
                db_r[:, :, 0 : W - 1],
                db_r[:, :, 0 : W - 1],
                AF.Square,
                accum_out=acc[:, COL_DX + c : COL_DX + c + 1],
            )
            db2 = flp.tile([HP, N_RECON], F16, tag="fscr", name=f"dz{c}")[:]
            nc.vector.tensor_sub(
                db2, fc_flat[:, W : W + N_RECON], fc_flat[:, 0:N_RECON]
            )
            nc.scalar.activation(
                db2, db2, AF.Square, accum_out=acc[:, COL_DZ + c : COL_DZ + c + 1]
            )

        def mse():
            md = flp.tile([HP, N_RECON], F16, tag="fscr", name="mse")[:]
            nc.vector.tensor_sub(
                md,
                inJ_r[:, 4 : 4 + DQ, :].rearrange("p d w -> p (d w)"),
                recon,
            )
            nc.scalar.activation(
                md, md, AF.Square, accum_out=acc[:, COL_MSE : COL_MSE + 1]
            )

        # -------- software-pipelined emission: pass2(v) zipped with pass1(v+1)
        boxes = {}
        srcs = {"J": inJ_r, "I": inI_r}
        # all products up-front: II on gpsimd, JJ/IJ fill DVE's idle head
        for v in ("II", "JJ", "IJ"):
            srcs[v] = product(v)

        yt_rs, t3_rs = {}, {}

        def begin_var(v):
            ytt = ytp.tile([HP, N_YT], F16, tag="yt", name=f"yt{v}")[:]
            yt_rs[v] = ytt.rearrange("p (d h) -> p d h", h=HP)
            nc.gpsimd.memset(yt_rs[v][:, D_IN:YT_R, :], 0.0)
            return pass1_chunks(v, srcs[v], yt_rs[v])

        def begin_pass2(v):
            t3t = t3p.tile([HP, N_T3], F16, tag="t3", name=f"t3{v}")[:]
            t3_rs[v] = t3t.rearrange("p (r h) -> p r h", h=HP)
            return pass2_chunks(v, yt_rs[v], t3_rs[v])

        pending_p2 = None
        pending_v = None
        flow_between = {"J": lambda: mse(), "I": lambda: flow_dy(0),
                        "II": lambda: (flow_dy(1), flow_dxz(0)),
                        "JJ": lambda: (flow_dxz(1),)}
        for v in VARS:
            p1 = begin_var(v)
            if pending_p2 is None:
                for e in p1:
                    e()
            else:
                for a, b in zip(pending_p2, p1):
                    a()
                    b()
                boxes[pending_v] = d_final(
                    pending_v, t3_rs[pending_v], gps=pending_v in ("J", "I")
                )
                fb = flow_between.get(pending_v)
                if fb:
                    fb()
            pending_p2 = begin_pass2(v)
            pending_v = v
        # last var (IJ): emit pass2 chunks; D-final+cc slices gated per chunk
        ij_p2 = pending_p2

        # IJ D-final per cc-slice, zipped with IJ's own pass2 chunks:
        # slice sl needs t3 rows 8sl..8sl+14 -> pass2 chunks 0..(sl//2+1)
        Bij = bxp.tile([HP, N_BOX], F16, tag="boxIJ", name="boxIJ")[:]
        Bij_r = Bij.rearrange("p (do h) -> p do h", h=HP)
        boxes["IJ"] = Bij
        t3ij = None

        NSL = 4
        NS = N_BOX // NSL
        emitted = 0

        def emit_ij_until(chunk):
            while emit_ij_until.ct <= chunk:
                ij_p2[emit_ij_until.ct]()
                emit_ij_until.ct += 1
        emit_ij_until.ct = 0

        def d_final_ij_slice(sl):
            do0 = 8 * sl
            t3r = t3_rs["IJ"]
            nc.vector.tensor_add(
                Bij_r[:, do0 : do0 + 8, :],
                t3r[:, do0 : do0 + 8, :],
                t3r[:, do0 + 3 : do0 + 11, :],
            )
            nc.vector.tensor_add(
                Bij_r[:, do0 : do0 + 8, :],
                Bij_r[:, do0 : do0 + 8, :],
                t3r[:, do0 + 6 : do0 + 14, :],
            )

        flow_dy(2)

        pend = None
        for sl in range(NSL):
            emit_ij_until(min(sl + 1, 4))
            d_final_ij_slice(sl)
            lo, hi = sl * NS, (sl + 1) * NS
            s1 = scp.tile([HP, NS], F16, tag="s1", name="s1")[:]
            s2 = scp.tile([HP, NS], F16, tag="s2", name="s2")[:]
            s3 = scp.tile([HP, NS], F16, tag="s3", name="s3")[:]
            bi, bj = boxes["I"][:, lo:hi], boxes["J"][:, lo:hi]
            bii, bjj = boxes["II"][:, lo:hi], boxes["JJ"][:, lo:hi]
            bij = boxes["IJ"][:, lo:hi]

            nc.vector.tensor_mul(s1, bi, bj)
            nc.vector.tensor_sub(s2, bij, s1)        # crossS
            nc.vector.tensor_mul(s1, bi, bi)
            nc.vector.tensor_sub(s3, bii, s1)        # IvarS
            nc.vector.tensor_mul(s1, bj, bj)
            nc.vector.tensor_sub(s1, bjj, s1)        # JvarS
            nc.vector.tensor_mul(s1, s1, s3)         # denomS
            nc.vector.tensor_mul(s3, s2, s2)         # crossS^2
            if pend is not None:
                ps1_, ps2_, ps3_, pcol = pend
                nc.vector.tensor_sub(ps1_, ps2_, ps3_)
                nc.scalar.activation(
                    ps3_, ps1_, AF.Exp, accum_out=acc[:, pcol : pcol + 1]
                )
            nc.scalar.activation(s2, s3, AF.Ln)      # ln cross^2
            nc.scalar.activation(s3, s1, AF.Ln, bias=eps_ap)
            pend = (s1, s2, s3, COL_CC + sl)
            if sl == 1:
                flow_dxz(2)
        ps1_, ps2_, ps3_, pcol = pend
        nc.vector.tensor_sub(ps1_, ps2_, ps3_)
        nc.scalar.activation(ps3_, ps1_, AF.Exp, accum_out=acc[:, pcol : pcol + 1])

        nc.sync.dma_start(out=d_out, in_=acc)

    nc.compile()
    return nc


def _make_consts():
    k = np.arange(HP)
    band = (np.abs(k[:, None] - k[None, :]) <= 4).astype(np.float16)
    bandq = (band * np.float16(S16)).astype(np.float16)
    bandl = (band * np.float16(TLIN)).astype(np.float16)
    m = np.arange(HP - 1)
    bidiag = np.zeros((HP, HP - 1), np.float16)
    bidiag[m + 1, m] = 1.0
    bidiag[m, m] = -1.0
    return band, bandq, bandl, bidiag


def _shard_inputs(imgsA, recon_A, warped_BA, flow_BA):
    bandh, bandq, bandl, bidiag = _make_consts()
    in_maps = []
    for core in range(8):
        b, q = divmod(core, 4)
        d0 = DQ * q

        def slab(vol):
            s = np.zeros((HP, D_IN, W), np.float16)
            lo, hi = d0 - 4, d0 + DQ + 4
            clo, chi = max(lo, 0), min(hi, D_FULL)
            s[:, clo - lo : chi - lo, :] = vol[clo:chi].transpose(1, 0, 2)
            return s.reshape(HP, N_IN)

        rec = (
            recon_A[b, 0, d0 : d0 + DQ]
            .transpose(1, 0, 2)
            .astype(np.float16)
            .reshape(HP, N_RECON)
        )

        fl = np.empty((HP, 3, FLOW_D, W), np.float16)
        hi = min(d0 + FLOW_D, D_FULL)
        n = hi - d0
        fl[:, :, :n] = flow_BA[b, :, d0:hi].transpose(2, 0, 1, 3)
        if n < FLOW_D:
            fl[:, :, n:] = fl[:, :, n - 1 : n]

        in_maps.append(
            {
                "inI": slab(warped_BA[b, 0]),
                "inJ": slab(imgsA[b, 0]),
                "recon": np.ascontiguousarray(rec),
                "flow": np.ascontiguousarray(fl).reshape(HP, 3 * N_FLOW_C),
                "bandh": bandh,
                "bandq": bandq,
                "bandl": bandl,
                "bidiag": bidiag,
            }
        )
    return in_maps


def _install_profile_shim():
    """Wire up NTFF profiling under axon when antenv.axon_hooks is absent."""
    try:
        import antenv.axon_hooks  # noqa: F401

        return True
    except ImportError:
        pass
    import contextlib
    import ctypes
    import types

    so_path = "/opt/axon/libaxon_pjrt.so"
    if not os.path.exists(so_path):
        return False
    lib = ctypes.CDLL(so_path)
    if not hasattr(lib, "axon_start_nrt_profile"):
        return False
    lib.axon_start_nrt_profile.argtypes = [
        ctypes.POINTER(ctypes.c_int64),
        ctypes.c_size_t,
    ]
    lib.axon_start_nrt_profile.restype = ctypes.c_int64
    lib.axon_stop_nrt_profile.argtypes = [ctypes.c_char_p]
    lib.axon_stop_nrt_profile.restype = ctypes.c_int64

    @contextlib.contextmanager
    def _hook(output_dir, device_ids):
        import jax

        jax.devices()
        if device_ids:
            ids = (ctypes.c_int64 * len(device_ids))(*device_ids)
            rc = lib.axon_start_nrt_profile(ids, len(device_ids))
        else:
            rc = lib.axon_start_nrt_profile(None, 0)
        if rc != 0:
            raise RuntimeError(f"axon_start_nrt_profile rc={rc}")
        try:
            yield
        finally:
            n = lib.axon_stop_nrt_profile(str(output_dir).encode())
            print(f"ntff profile: {n} file(s) written to {output_dir}")

    mod = types.ModuleType("antenv.axon_hooks")
    mod.get_axon_ntff_profile_hook = lambda: _hook
    mod.set_axon_ntff_profile_hook = lambda h: None
    import antenv

    sys.modules["antenv.axon_hooks"] = mod
    antenv.axon_hooks = mod

    import concourse.bass_utils as _bu

    _bu.upload_artifacts = lambda tmpdir: tmpdir
    return True


LAST_EXEC_NS = None
LAST_RESULTS = None


def kernel(imgsA, recon_A, warped_BA, flow_BA):
    global LAST_EXEC_NS, LAST_RESULTS
    if "nc" not in _CACHE:
        _CACHE["nc"] = _build_program()
    nc = _CACHE["nc"]

    in_maps = _shard_inputs(
        np.asarray(imgsA, np.float32),
        np.asarray(recon_A, np.float32),
        np.asarray(warped_BA, np.float32),
        np.asarray(flow_BA, np.float32),
    )
    trace = os.environ.get("GVSL_TRACE", "0") == "1"
    if trace:
        trace = _install_profile_shim()
    tmpdir = os.environ.get("GVSL_TRACE_DIR") or None
    res = run_bass_kernel_spmd(
        nc, in_maps, core_ids=list(range(8)), trace=trace, tmpdir=tmpdir
    )
    LAST_EXEC_NS = res.exec_time_ns
    LAST_RESULTS = res

    cc = mse_s = dx = dy = dz = 0.0
    for r in res.results:
        o = np.asarray(r["out"], np.float64)
        cc += o[:, COL_CC : COL_CC + 4].sum()
        mse_s += o[:, COL_MSE].sum()
        dx += o[:, COL_DX : COL_DX + 3].sum()
        dz += o[:, COL_DZ : COL_DZ + 3].sum()
        dy += o[: HP - 1, COL_DY : COL_DY + 12].sum()

    if os.environ.get("GVSL_DEBUG_COLS"):
        tot = np.zeros(ACC_W)
        for r in res.results:
            tot += np.asarray(r["out"], np.float64).sum(axis=0)
        n_dd = 2.0 * 127 * 128 * 128
        print("cols cc:", tot[COL_CC : COL_CC + 4])
        print("col mse:", tot[COL_MSE])
        print("cols dx/nd:", tot[COL_DX : COL_DX + 3] / n_dd * 3)
        print("cols dz/nd:", tot[COL_DZ : COL_DZ + 3] / n_dd * 3)
        print("cols dy/nd:", tot[COL_DY : COL_DY + 12].reshape(3, 4) / n_dd * 3)

    n_vox = 2 * 1 * 128 * 128 * 128
    n_d = 2 * 3 * 127 * 128 * 128
    ncc_loss = 1.0 - cc / n_vox
    mse_loss = mse_s / n_vox
    smooth_loss = (dx / n_d + dy / n_d + dz / n_d) / 3.0
    return (
        np.float32(ncc_loss),
        np.float32(mse_loss),
        np.float32(smooth_loss),
    )


# revision 20
# speedup vs baseline: 1.1359x; 1.1359x over previous
"""GVSL loss (NCC + MSE + smoothness) as a distributed Bass kernel on 8 TRN2 cores.

Sharding: batch(2) x depth-quarters(4) = 8 shards; each core owns a 32-deep
output slab (+4-voxel halo for the 9^3 box filter).

NCC box filter strategy (per var in {I, J, I^2, J^2, IJ}):
  pass1 (PE):  per d-row matmul(lhsT=V_d[h,w], rhs=BandH[h,h']) -> PSUM [w, h]
               = H-box + transpose in one shot (fp16, FD=128, LDW pipelined)
  evac1:       PSUM -> SBUF fp16 YT [w, (d, h)]
  pass2 (PE):  stationary BandW (scaled); 3 d-shifted FD=512 matmuls accumulate
               -> t3[r] = Z[r]+Z[r+1]+Z[r+2] (W-box + D-triple), PSUM
  evac2:       PSUM -> SBUF fp16 T3 [w', (r, h)]
  D-final(DVE): S = t3[d] + t3[d+3] + t3[d+6]  (9-window box done)

Scaling: quadratic vars (II/JJ/IJ) get BandW*s (s=fp16(1/27)); linear vars
(I/J) get BandW*t with t^2 = s/729, so cross = B'IJ - B'I*B'J and
var = B'II - B'I^2 need no scalar coefficient (all plain tensor ops), and
everything stays in fp16 range. eps scales as 1e-5*s^2.

The depth range is processed in two phases (A: do 0..15, B: do 16..31) so the
cc math for A overlaps phase B's box-filter work.
"""

import os
import sys

for _p in ("/opt/trn_rl_repo",):
    if _p not in sys.path:
        sys.path.insert(0, _p)

import numpy as np

import concourse.bass as bass
import concourse.tile as tile
from concourse import bacc, mybir
from concourse.bass_utils import run_bass_kernel_spmd

F32 = mybir.dt.float32
F16 = mybir.dt.float16
AF = mybir.ActivationFunctionType
ALU = mybir.AluOpType

HP = 128
W = 128
D_FULL = 128
DQ = 32
D_IN = DQ + 8     # 40 slab rows incl halo
YT_R = 42         # yt rows incl zero tail (pass2 k=9 s=2 reads rows 38..41)
T3_R = 40
FLOW_D = DQ + 1   # 33

N_IN = D_IN * W           # 5120
N_YT = YT_R * HP          # 5376
N_T3 = T3_R * HP          # 5120
N_BOX = DQ * HP           # 4096
N_RECON = DQ * W          # 4096
N_FLOW_C = FLOW_D * W     # 4224

S16 = float(np.float16(1.0 / 27.0))          # quadratic-var scale
TLIN = float(np.float16(np.sqrt(S16 / 729.0)))  # linear-var scale
EPS_S = 1e-5 * S16 * S16

COL_CC = 0     # +4
COL_MSE = 4
COL_DX = 5     # +3
COL_DZ = 8     # +3
COL_DY = 11    # +12
ACC_W = 24

VARS = ("J", "I", "II", "JJ", "IJ")

_CACHE = {}


def _patch_act_tables():
    """Reorder activation-table sets so the one containing ln+exp+square+copy
    is preferred, avoiding table reloads between Ln and Exp/Square."""
    from concourse import hw_specs

    if getattr(hw_specs, "_gvsl_patched", False):
        return
    orig = hw_specs.get_activation_tables

    def patched(arch):
        t = dict(orig(arch))
        key = "natural_log_exp_and_others"
        if key in t:
            t = {key: t[key], **{k: v for k, v in t.items() if k != key}}
        return t

    hw_specs.get_activation_tables = patched
    bacc.get_activation_tables = patched
    hw_specs._gvsl_patched = True


def _build_program():
    nc = bacc.Bacc("TRN2", target_bir_lowering=False, debug=False, num_devices=8)

    d_inI = nc.dram_tensor("inI", [HP, N_IN], F16, kind="ExternalInput").ap()
    d_inJ = nc.dram_tensor("inJ", [HP, N_IN], F16, kind="ExternalInput").ap()
    d_recon = nc.dram_tensor("recon", [HP, N_RECON], F16, kind="ExternalInput").ap()
    d_flow = nc.dram_tensor("flow", [HP, 3 * N_FLOW_C], F16, kind="ExternalInput").ap()
    d_bandh = nc.dram_tensor("bandh", [HP, HP], F16, kind="ExternalInput").ap()
    d_bandq = nc.dram_tensor("bandq", [HP, HP], F16, kind="ExternalInput").ap()
    d_bandl = nc.dram_tensor("bandl", [HP, HP], F16, kind="ExternalInput").ap()
    d_bidiag = nc.dram_tensor("bidiag", [HP, HP - 1], F16, kind="ExternalInput").ap()
    d_out = nc.dram_tensor("out", [HP, ACC_W], F32, kind="ExternalOutput").ap()

    from contextlib import ExitStack

    with tile.TileContext(nc) as tc, ExitStack() as es:
        pp = es.enter_context(tc.tile_pool(name="persist", bufs=1))
        prp = es.enter_context(tc.tile_pool(name="prodp", bufs=3))
        ytp = es.enter_context(tc.tile_pool(name="ytp", bufs=2))
        t3p = es.enter_context(tc.tile_pool(name="t3p", bufs=2))
        bxp = es.enter_context(tc.tile_pool(name="boxp", bufs=1))
        scp = es.enter_context(tc.tile_pool(name="scrp", bufs=2))
        flp = es.enter_context(tc.tile_pool(name="flowscr", bufs=2))
        ps1 = es.enter_context(tc.tile_pool(name="psum1", bufs=2, space="PSUM"))
        ps2 = es.enter_context(tc.tile_pool(name="psum2", bufs=2, space="PSUM"))

        acc = pp.tile([HP, ACC_W], F32, tag="acc", name="acc")[:]
        eps_ap = pp.tile([HP, 1], F32, tag="epsc", name="epsc")[:]
        nc.gpsimd.memset(eps_ap, EPS_S)

        bandh = pp.tile([HP, HP], F16, tag="bandh", name="bandh")[:]
        bandq = pp.tile([HP, HP], F16, tag="bandq", name="bandq")[:]
        bandl = pp.tile([HP, HP], F16, tag="bandl", name="bandl")[:]
        bidiag = pp.tile([HP, HP - 1], F16, tag="bidiag", name="bidiag")[:]
        inI = pp.tile([HP, N_IN], F16, tag="inI", name="inI")[:]
        inJ = pp.tile([HP, N_IN], F16, tag="inJ", name="inJ")[:]
        recon = pp.tile([HP, N_RECON], F16, tag="recon", name="recon")[:]
        flow = pp.tile([HP, 3 * N_FLOW_C], F16, tag="flow", name="flow")[:]

        nc.sync.dma_start(out=bandh, in_=d_bandh)
        nc.sync.dma_start(out=bandq, in_=d_bandq)
        nc.sync.dma_start(out=bandl, in_=d_bandl)
        nc.sync.dma_start(out=bidiag, in_=d_bidiag)
        NQ = N_IN // 4
        for c in range(4):
            nc.sync.dma_start(out=inJ[:, NQ * c : NQ * (c + 1)],
                              in_=d_inJ[:, NQ * c : NQ * (c + 1)])
        for c in range(4):
            nc.sync.dma_start(out=inI[:, NQ * c : NQ * (c + 1)],
                              in_=d_inI[:, NQ * c : NQ * (c + 1)])
        nc.sync.dma_start(out=recon, in_=d_recon)
        nc.sync.dma_start(out=flow, in_=d_flow)

        inI_r = inI.rearrange("p (d w) -> p d w", w=W)
        inJ_r = inJ.rearrange("p (d w) -> p d w", w=W)

        evac_ct = [0]

        def evac(dst, src):
            # rotate PSUM->SBUF copies across DVE / ACT / GPSIMD
            if evac_ct[0] % 2 == 0:
                nc.vector.tensor_copy(dst, src)
            else:
                nc.scalar.copy(dst, src)
            evac_ct[0] += 1

        def product(v):
            prod = prp.tile([HP, N_IN], F16, tag="prod", name=f"prod{v}")[:]
            if v == "II":
                nc.gpsimd.tensor_mul(prod, inI, inI)
            elif v == "JJ":
                nc.vector.tensor_mul(prod, inJ, inJ)
            else:
                nc.vector.tensor_mul(prod, inI, inJ)
            return prod.rearrange("p (d w) -> p d w", w=W)

        def pass1_chunks(v, src_r, yt_r):
            def mk(g0):
                def emit():
                    pst = ps1.tile([HP, 1024], F32, tag="ps1", name="ps1")[:]
                    for q in range(8):
                        nc.tensor.matmul(
                            pst[:, 128 * q : 128 * (q + 1)],
                            src_r[:, g0 + q, :],
                            bandh,
                            start=True,
                            stop=True,
                        )
                    dst = yt_r[:, g0 : g0 + 8, :].rearrange("p d h -> p (d h)")
                    evac(dst, pst)
                return emit
            return [mk(g0) for g0 in range(0, D_IN, 8)]

        def pass2_chunks(v, yt_r, t3_r):
            bw = bandl if v in ("I", "J") else bandq
            def mk(k0):
                def emit():
                    pst = ps2.tile([HP, 1024], F32, tag="ps2", name="ps2")[:]
                    for ki in range(2):
                        k = k0 + ki
                        for s in range(3):
                            rhs = yt_r[:, 4 * k + s : 4 * k + s + 4, :].rearrange(
                                "p d h -> p (d h)"
                            )
                            nc.tensor.matmul(
                                pst[:, 512 * ki : 512 * (ki + 1)],
                                bw,
                                rhs,
                                start=(s == 0),
                                stop=(s == 2),
                            )
                    dst = t3_r[:, 4 * k0 : 4 * k0 + 8, :].rearrange("p r h -> p (r h)")
                    evac(dst, pst)
                return emit
            return [mk(k0) for k0 in range(0, 10, 2)]

        def d_final(v, t3_r, gps=False):
            eng = nc.gpsimd if gps else nc.vector
            B = bxp.tile([HP, N_BOX], F16, tag=f"box{v}", name=f"box{v}")[:]
            B_r = B.rearrange("p (do h) -> p do h", h=HP)
            eng.tensor_add(B_r, t3_r[:, 0:DQ, :], t3_r[:, 3 : 3 + DQ, :])
            eng.tensor_add(B_r, B_r, t3_r[:, 6 : 6 + DQ, :])
            return B

        flow_r = flow.rearrange("p (c d w) -> p c d w", c=3, w=W)

        def flow_dy(c):
            fc_flat = flow_r[:, c].rearrange("p d w -> p (d w)")
            for half in range(2):
                pst = ps1.tile([HP, 1024], F32, tag="ps1", name="dy")[:]
                for j in range(2):
                    off = 2048 * half + 1024 * j
                    for jj in range(2):
                        nc.tensor.matmul(
                            pst[0 : HP - 1, 512 * jj : 512 * (jj + 1)],
                            bidiag,
                            fc_flat[:, off + 512 * jj : off + 512 * (jj + 1)],
                            start=True,
                            stop=True,
                        )
                    col = COL_DY + 4 * c + 2 * half + j
                    nc.scalar.activation(
                        pst[0 : HP - 1, :],
                        pst[0 : HP - 1, :],
                        AF.Square,
                        accum_out=acc[0 : HP - 1, col : col + 1],
                    )

        def flow_dxz(c):
            fc_flat = flow_r[:, c].rearrange("p d w -> p (d w)")
            db = flp.tile([HP, N_RECON], F16, tag="fscr", name=f"dx{c}")[:]
            nc.vector.tensor_sub(
                db, fc_flat[:, 1 : 1 + N_RECON], fc_flat[:, 0:N_RECON]
            )
            db_r = db.rearrange("p (d w) -> p d w", w=W)
            nc.scalar.activation(
                db_r[:, :, 0 : W - 1],
                db_r[:, :, 0 : W - 1],
                AF.Square,
                accum_out=acc[:, COL_DX + c : COL_DX + c + 1],
            )
            db2 = flp.tile([HP, N_RECON], F16, tag="fscr", name=f"dz{c}")[:]
            nc.vector.tensor_sub(
                db2, fc_flat[:, W : W + N_RECON], fc_flat[:, 0:N_RECON]
            )
            nc.scalar.activation(
                db2, db2, AF.Square, accum_out=acc[:, COL_DZ + c : COL_DZ + c + 1]
            )

        def mse():
            md = flp.tile([HP, N_RECON], F16, tag="fscr", name="mse")[:]
            nc.vector.tensor_sub(
                md,
                inJ_r[:, 4 : 4 + DQ, :].rearrange("p d w -> p (d w)"),
                recon,
            )
            nc.scalar.activation(
                md, md, AF.Square, accum_out=acc[:, COL_MSE : COL_MSE + 1]
            )

        # -------- software-pipelined emission: pass2(v) zipped with pass1(v+1)
        boxes = {}
        srcs = {"J": inJ_r, "I": inI_r}
        # all products up-front: II on gpsimd, JJ/IJ fill DVE's idle head
        for v in ("II", "JJ", "IJ"):
            srcs[v] = product(v)

        yt_rs, t3_rs = {}, {}

        def begin_var(v):
            ytt = ytp.tile([HP, N_YT], F16, tag="yt", name=f"yt{v}")[:]
            yt_rs[v] = ytt.rearrange("p (d h) -> p d h", h=HP)
            nc.gpsimd.memset(yt_rs[v][:, D_IN:YT_R, :], 0.0)
            return pass1_chunks(v, srcs[v], yt_rs[v])

        def begin_pass2(v):
            t3t = t3p.tile([HP, N_T3], F16, tag="t3", name=f"t3{v}")[:]
            t3_rs[v] = t3t.rearrange("p (r h) -> p r h", h=HP)
            return pass2_chunks(v, yt_rs[v], t3_rs[v])

        pending_p2 = None
        pending_v = None
        flow_between = {"J": lambda: mse(), "I": lambda: flow_dy(0),
                        "II": lambda: (flow_dy(1), flow_dxz(0)),
                        "JJ": lambda: (flow_dxz(1),)}
        for v in VARS:
            p1 = begin_var(v)
            if pending_p2 is None:
                for e in p1:
                    e()
            else:
                for a, b in zip(pending_p2, p1):
                    a()
                    b()
                boxes[pending_v] = d_final(pending_v, t3_rs[pending_v])
                fb = flow_between.get(pending_v)
                if fb:
                    fb()
            pending_p2 = begin_pass2(v)
            pending_v = v
        # last var (IJ): emit pass2 chunks; D-final+cc slices gated per chunk
        ij_p2 = pending_p2

        # IJ D-final per cc-slice, zipped with IJ's own pass2 chunks:
        # slice sl needs t3 rows 8sl..8sl+14 -> pass2 chunks 0..(sl//2+1)
        Bij = bxp.tile([HP, N_BOX], F16, tag="boxIJ", name="boxIJ")[:]
        Bij_r = Bij.rearrange("p (do h) -> p do h", h=HP)
        boxes["IJ"] = Bij
        t3ij = None

        NSL = 4
        NS = N_BOX // NSL
        emitted = 0

        def emit_ij_until(chunk):
            while emit_ij_until.ct <= chunk:
                ij_p2[emit_ij_until.ct]()
                emit_ij_until.ct += 1
        emit_ij_until.ct = 0

        def d_final_ij_slice(sl):
            do0 = 8 * sl
            t3r = t3_rs["IJ"]
            nc.vector.tensor_add(
                Bij_r[:, do0 : do0 + 8, :],
                t3r[:, do0 : do0 + 8, :],
                t3r[:, do0 + 3 : do0 + 11, :],
            )
            nc.vector.tensor_add(
                Bij_r[:, do0 : do0 + 8, :],
                Bij_r[:, do0 : do0 + 8, :],
                t3r[:, do0 + 6 : do0 + 14, :],
            )

        flow_dy(2)

        pend = None
        for sl in range(NSL):
            emit_ij_until(min(sl + 1, 4))
            d_final_ij_slice(sl)
            lo, hi = sl * NS, (sl + 1) * NS
            s1 = scp.tile([HP, NS], F16, tag="s1", name="s1")[:]
            s2 = scp.tile([HP, NS], F16, tag="s2", name="s2")[:]
            s3 = scp.tile([HP, NS], F16, tag="s3", name="s3")[:]
            bi, bj = boxes["I"][:, lo:hi], boxes["J"][:, lo:hi]
            bii, bjj = boxes["II"][:, lo:hi], boxes["JJ"][:, lo:hi]
            bij = boxes["IJ"][:, lo:hi]

            nc.vector.tensor_mul(s1, bi, bj)
            nc.vector.tensor_sub(s2, bij, s1)        # crossS
            nc.vector.tensor_mul(s1, bi, bi)
            nc.vector.tensor_sub(s3, bii, s1)        # IvarS
            nc.vector.tensor_mul(s1, bj, bj)
            nc.vector.tensor_sub(s1, bjj, s1)        # JvarS
            nc.vector.tensor_mul(s1, s1, s3)         # denomS
            nc.vector.tensor_mul(s3, s2, s2)         # crossS^2
            if pend is not None:
                ps1_, ps2_, ps3_, pcol = pend
                nc.vector.tensor_sub(ps1_, ps2_, ps3_)
                nc.scalar.activation(
                    ps3_, ps1_, AF.Exp, accum_out=acc[:, pcol : pcol + 1]
                )
            nc.scalar.activation(s2, s3, AF.Ln)      # ln cross^2
            nc.scalar.activation(s3, s1, AF.Ln, bias=eps_ap)
            pend = (s1, s2, s3, COL_CC + sl)
            if sl == 1:
                flow_dxz(2)
        ps1_, ps2_, ps3_, pcol = pend
        nc.vector.tensor_sub(ps1_, ps2_, ps3_)
        nc.scalar.activation(ps3_, ps1_, AF.Exp, accum_out=acc[:, pcol : pcol + 1])

        nc.sync.dma_start(out=d_out, in_=acc)

    nc.compile()
    return nc


def _make_consts():
    k = np.arange(HP)
    band = (np.abs(k[:, None] - k[None, :]) <= 4).astype(np.float16)
    bandq = (band * np.float16(S16)).astype(np.float16)
    bandl = (band * np.float16(TLIN)).astype(np.float16)
    m = np.arange(HP - 1)
    bidiag = np.zeros((HP, HP - 1), np.float16)
    bidiag[m + 1, m] = 1.0
    bidiag[m, m] = -1.0
    return band, bandq, bandl, bidiag


def _shard_inputs(imgsA, recon_A, warped_BA, flow_BA):
    bandh, bandq, bandl, bidiag = _make_consts()
    in_maps = []
    for core in range(8):
        b, q = divmod(core, 4)
        d0 = DQ * q

        def slab(vol):
            s = np.zeros((HP, D_IN, W), np.float16)
            lo, hi = d0 - 4, d0 + DQ + 4
            clo, chi = max(lo, 0), min(hi, D_FULL)
            s[:, clo - lo : chi - lo, :] = vol[clo:chi].transpose(1, 0, 2)
            return s.reshape(HP, N_IN)

        rec = (
            recon_A[b, 0, d0 : d0 + DQ]
            .transpose(1, 0, 2)
            .astype(np.float16)
            .reshape(HP, N_RECON)
        )

        fl = np.empty((HP, 3, FLOW_D, W), np.float16)
        hi = min(d0 + FLOW_D, D_FULL)
        n = hi - d0
        fl[:, :, :n] = flow_BA[b, :, d0:hi].transpose(2, 0, 1, 3)
        if n < FLOW_D:
            fl[:, :, n:] = fl[:, :, n - 1 : n]

        in_maps.append(
            {
                "inI": slab(warped_BA[b, 0]),
                "inJ": slab(imgsA[b, 0]),
                "recon": np.ascontiguousarray(rec),
                "flow": np.ascontiguousarray(fl).reshape(HP, 3 * N_FLOW_C),
                "bandh": bandh,
                "bandq": bandq,
                "bandl": bandl,
                "bidiag": bidiag,
            }
        )
    return in_maps


def _install_profile_shim():
    """Wire up NTFF profiling under axon when antenv.axon_hooks is absent."""
    try:
        import antenv.axon_hooks  # noqa: F401

        return True
    except ImportError:
        pass
    import contextlib
    import ctypes
    import types

    so_path = "/opt/axon/libaxon_pjrt.so"
    if not os.path.exists(so_path):
        return False
    lib = ctypes.CDLL(so_path)
    if not hasattr(lib, "axon_start_nrt_profile"):
        return False
    lib.axon_start_nrt_profile.argtypes = [
        ctypes.POINTER(ctypes.c_int64),
        ctypes.c_size_t,
    ]
    lib.axon_start_nrt_profile.restype = ctypes.c_int64
    lib.axon_stop_nrt_profile.argtypes = [ctypes.c_char_p]
    lib.axon_stop_nrt_profile.restype = ctypes.c_int64

    @contextlib.contextmanager
    def _hook(output_dir, device_ids):
        import jax

        jax.devices()
        if device_ids:
            ids = (ctypes.c_int64 * len(device_ids))(*device_ids)
            rc = lib.axon_start_nrt_profile(ids, len(device_ids))
        else:
            rc = lib.axon_start_nrt_profile(None, 0)
        if rc != 0:
            raise RuntimeError(f"axon_start_nrt_profile rc={rc}")
        try:
            yield
        finally:
            n = lib.axon_stop_nrt_profile(str(output_dir).encode())
            print(f"ntff profile: {n} file(s) written to {output_dir}")

    mod = types.ModuleType("antenv.axon_hooks")
    mod.get_axon_ntff_profile_hook = lambda: _hook
    mod.set_axon_ntff_profile_hook = lambda h: None
    import antenv

    sys.modules["antenv.axon_hooks"] = mod
    antenv.axon_hooks = mod

    import concourse.bass_utils as _bu

    _bu.upload_artifacts = lambda tmpdir: tmpdir
    return True


LAST_EXEC_NS = None
LAST_RESULTS = None


def kernel(imgsA, recon_A, warped_BA, flow_BA):
    global LAST_EXEC_NS, LAST_RESULTS
    if "nc" not in _CACHE:
        _CACHE["nc"] = _build_program()
    nc = _CACHE["nc"]

    in_maps = _shard_inputs(
        np.asarray(imgsA, np.float32),
        np.asarray(recon_A, np.float32),
        np.asarray(warped_BA, np.float32),
        np.asarray(flow_BA, np.float32),
    )
    trace = os.environ.get("GVSL_TRACE", "0") == "1"
    if trace:
        trace = _install_profile_shim()
    tmpdir = os.environ.get("GVSL_TRACE_DIR") or None
    res = run_bass_kernel_spmd(
        nc, in_maps, core_ids=list(range(8)), trace=trace, tmpdir=tmpdir
    )
    LAST_EXEC_NS = res.exec_time_ns
    LAST_RESULTS = res

    cc = mse_s = dx = dy = dz = 0.0
    for r in res.results:
        o = np.asarray(r["out"], np.float64)
        cc += o[:, COL_CC : COL_CC + 4].sum()
        mse_s += o[:, COL_MSE].sum()
        dx += o[:, COL_DX : COL_DX + 3].sum()
        dz += o[:, COL_DZ : COL_DZ + 3].sum()
        dy += o[: HP - 1, COL_DY : COL_DY + 12].sum()

    if os.environ.get("GVSL_DEBUG_COLS"):
        tot = np.zeros(ACC_W)
        for r in res.results:
            tot += np.asarray(r["out"], np.float64).sum(axis=0)
        n_dd = 2.0 * 127 * 128 * 128
        print("cols cc:", tot[COL_CC : COL_CC + 4])
        print("col mse:", tot[COL_MSE])
        print("cols dx/nd:", tot[COL_DX : COL_DX + 3] / n_dd * 3)
        print("cols dz/nd:", tot[COL_DZ : COL_DZ + 3] / n_dd * 3)
        print("cols dy/nd:", tot[COL_DY : COL_DY + 12].reshape(3, 4) / n_dd * 3)

    n_vox = 2 * 1 * 128 * 128 * 128
    n_d = 2 * 3 * 127 * 128 * 128
    ncc_loss = 1.0 - cc / n_vox
    mse_loss = mse_s / n_vox
    smooth_loss = (dx / n_d + dy / n_d + dz / n_d) / 3.0
    return (
        np.float32(ncc_loss),
        np.float32(mse_loss),
        np.float32(smooth_loss),
    )
